# revision 3
# baseline (speedup 1.0000x reference)
"""Causal cosine-sim attention (qk rmsnorm, scale=8) on 8 trn2 NeuronCores.

Shapes: q,k,v [2,16,2048,64] fp32; out [2,16,2048,64] fp32.
Sharding: 32 (batch, head) pairs -> 4 per core (head-parallel); each core
runs an identical SPMD program on its own 4 heads.

Per-core flash-attention-style algorithm (per head):
  preprocess (per 1024-token half, so the first matmul starts early and
    the next half overlaps attention): load Q (sync DMA) / K (gpsimd
    SWDGE); l2-normalize rows (squares on DVE for q / GpSimd for k,
    reduce + Quake-rsqrt + 2 Newton iters on DVE); cast bf16; round-trip
    through a DRAM scratch [1024,64] and DMA-xbar-transpose back TWICE
    into both 64-partition halves of qT/kT [128, s] (duplicated
    partition halves make the S^T matmuls contract over K=128: K=64
    matmuls don't count as PE-busy for the HAM activity monitor and pin
    the PE clock at 1.2 GHz).  q_scale*k_scale is folded into qT only.
    V loads [s,d] (gpsimd queue), cast bf16 with a ones-column appended
    (rowsum rides along in the PV matmul).
  attention (j-major over key blocks, i-halves of 1024): per key block
    jb, one S^T tile = kT_jb.T @ qT over the causal i-tail (PSUM,
    512-col matmuls); on diagonal blocks a constant strict-upper
    -1e30 matrix is matmul-accumulated into the first 128 cols BEFORE
    the exp (tri add via PE, so no GpSimd mask multiply serializes the
    PV chain); one ACT exp(4*x + mask_bias) per jb PSUM->SBUF bf16;
    O^T[65, i-half] += V_jb.T @ P^T accumulates in PSUM (col 64 =
    softmax denominator).  PV_{jb-1} is emitted AFTER QK_jb so the PE
    FIFO never head-of-line blocks on ScalarE's exp.
  epilogue per half: O^T PSUM->SBUF copy immediately (frees the PSUM
    bank for the next half); transposes (PE) + rowsum divide (DVE) +
    out-DMA (gpsimd queue) are deferred until the next half's pipeline
    is running so the PE never idles at half boundaries.
"""

import sys

import numpy as np

try:
    import concourse.bass as bass
except ImportError:
    sys.path.insert(0, "/opt/trn_rl_repo")
    import concourse.bass as bass

import concourse.mybir as mybir
import concourse.tile as tile
from concourse import bacc
from concourse.bass_utils import run_bass_kernel_spmd
from concourse.masks import make_identity

FP32 = mybir.dt.float32
BF16 = mybir.dt.bfloat16

N_CORES = 8
B, H, S, D = 2, 16, 2048, 64
HPC = (B * H) // N_CORES  # heads per core = 4
P = 128
NT = S // P  # 16 key/query blocks
HALF = S // 2
NTH = HALF // P  # 8 blocks per half
COSINE_SIM_SCALE = 8.0
MASK_NEG = -1e30


def build_nc():
    nc = bacc.Bacc("TRN2", target_bir_lowering=False, debug=False)

    q_d = nc.dram_tensor("q", [HPC, S, D], FP32, kind="ExternalInput")
    k_d = nc.dram_tensor("k", [HPC, S, D], FP32, kind="ExternalInput")
    v_d = nc.dram_tensor("v", [HPC, S, D], FP32, kind="ExternalInput")
    qs_d = nc.dram_tensor("q_scale", [D], FP32, kind="ExternalInput")
    ks_d = nc.dram_tensor("k_scale", [D], FP32, kind="ExternalInput")
    mb_d = nc.dram_tensor("mbias", [HPC, S], FP32, kind="ExternalInput")
    out_d = nc.dram_tensor("out", [HPC, S, D], FP32, kind="ExternalOutput")

    AF = mybir.ActivationFunctionType
    ALU = mybir.AluOpType

    with tile.TileContext(nc) as tc:
        with (
            tc.tile_pool(name="constp", bufs=1) as constp,
            tc.tile_pool(name="dramp", bufs=2, space="DRAM") as dramp,
            tc.tile_pool(name="stagep", bufs=3) as stagep,
            tc.tile_pool(name="sqp", bufs=2) as sqp,
            tc.tile_pool(name="ssp", bufs=6) as ssp,
            tc.tile_pool(name="qnp", bufs=3) as qnp,
            tc.tile_pool(name="qtp", bufs=2) as qtp,
            tc.tile_pool(name="ktp", bufs=2) as ktp,
            tc.tile_pool(name="vbp", bufs=2) as vbp,
            tc.tile_pool(name="mbp", bufs=2) as mbp,
            tc.tile_pool(name="ptp", bufs=8) as ptp,
            tc.tile_pool(name="otsbp", bufs=2) as otsbp,
            tc.tile_pool(name="osbp", bufs=2) as osbp,
            tc.tile_pool(name="recp", bufs=8) as recp,
            tc.tile_pool(name="stp", bufs=3, space="PSUM") as stp,
            tc.tile_pool(name="otp", bufs=1, space="PSUM") as otp,
        ):
            # ---- constants ----
            ident = constp.tile([P, P], FP32, name="ident")
            make_identity(nc, ident[:])
            identb = constp.tile([P, P], BF16, name="identb")
            nc.vector.tensor_copy(identb[:], ident[:])
            # trineg[row=i, col=j] = -1e30 where j > i (strict upper).
            # Used as lhsT in a matmul-accumulate against identb:
            # st[j, i] += trineg[i, j] masks j > i before the exp.
            trineg = constp.tile([P, P], BF16, name="trineg")
            nc.gpsimd.memset(trineg[:], MASK_NEG)
            nc.gpsimd.affine_select(
                out=trineg[:],
                in_=trineg[:],
                pattern=[[1, P]],
                channel_multiplier=-1,
                base=-1,
                compare_op=ALU.is_ge,
                fill=0.0,
            )
            # combined q_scale*k_scale (applied to qT only), duplicated
            # over both partition halves to match the row-packed qT
            csc = constp.tile([P, 1], FP32, name="csc")
            kss = constp.tile([P, 1], FP32, name="kss")
            for half in range(2):
                nc.sync.dma_start(
                    out=csc[half * D : (half + 1) * D, 0:1],
                    in_=qs_d[:].rearrange("(d one) -> d one", one=1),
                )
                nc.sync.dma_start(
                    out=kss[half * D : (half + 1) * D, 0:1],
                    in_=ks_d[:].rearrange("(d one) -> d one", one=1),
                )
            nc.vector.tensor_mul(csc[:], csc[:], kss[:])

            # ============ per-(head, half) preprocess ============
            st8 = {}  # h -> dict(qT, kT, vb, mb)

            def pre_half(h, hf):
                c0 = hf * HALF
                tb = hf * NTH  # first 128-token block of this half
                if hf == 0:
                    st8[h] = {
                        "qT": qtp.tile([P, S], BF16, tag="qT", name=f"qT{h}"),
                        "kT": ktp.tile([P, S], BF16, tag="kT", name=f"kT{h}"),
                        "vb": vbp.tile(
                            [P, NT * (D + 1)], BF16, tag="vb", name=f"vb{h}"
                        ),
                        "mb": mbp.tile([P, NT], FP32, tag="mb", name=f"mb{h}"),
                    }
                    nc.sync.dma_start(
                        out=st8[h]["mb"][:],
                        in_=mb_d[h].rearrange("(t p) -> p t", p=P),
                    )
                sd = st8[h]

                xq = stagep.tile([P, NTH * D], FP32, tag="xq", name=f"xq{h}_{hf}")
                nc.sync.dma_start(
                    out=xq.rearrange("p (t d) -> p t d", d=D),
                    in_=q_d[h].rearrange("(t p) d -> p t d", p=P)[
                        :, tb : tb + NTH, :
                    ],
                )
                xk = stagep.tile([P, NTH * D], FP32, tag="xk", name=f"xk{h}_{hf}")
                nc.sync.dma_start(
                    out=xk.rearrange("p (t d) -> p t d", d=D),
                    in_=k_d[h].rearrange("(t p) d -> p t d", p=P)[
                        :, tb : tb + NTH, :
                    ],
                )
                # sum-of-squares: q on DVE, k squares on GpSimd (reduce
                # must be DVE: GpSimd cannot reduce along the free axis)
                sqq = sqp.tile([P, NTH * D], FP32, tag="sqq", name=f"sqq{h}_{hf}")
                nc.vector.tensor_mul(sqq[:], xq[:], xq[:])
                sqk = sqp.tile([P, NTH * D], FP32, tag="sqk", name=f"sqk{h}_{hf}")
                nc.gpsimd.tensor_mul(sqk[:], xk[:], xk[:])
                ss = ssp.tile([P, 2 * NTH], FP32, tag="ss", name=f"ss{h}_{hf}")
                nc.vector.tensor_reduce(
                    out=ss[:, 0:NTH],
                    in_=sqq.rearrange("p (t d) -> p t d", d=D),
                    axis=mybir.AxisListType.X,
                    op=ALU.add,
                )
                nc.vector.tensor_reduce(
                    out=ss[:, NTH : 2 * NTH],
                    in_=sqk.rearrange("p (t d) -> p t d", d=D),
                    axis=mybir.AxisListType.X,
                    op=ALU.add,
                )
                # rsqrt(ss) on DVE: Quake magic init + 2 Newton iterations
                rs = ssp.tile([P, 2 * NTH], FP32, tag="ss", name=f"rs{h}_{hf}")
                rsi = rs.bitcast(mybir.dt.int32)
                nc.vector.tensor_scalar(
                    rsi, ss.bitcast(mybir.dt.int32), 1, None, ALU.arith_shift_right
                )
                nc.vector.tensor_scalar(
                    rsi, rsi, -1.0, float(0x5F3759DF), ALU.mult, ALU.add
                )
                tnw = ssp.tile([P, 2 * NTH], FP32, tag="ss", name=f"tnw{h}_{hf}")
                for _ in range(2):
                    nc.vector.tensor_mul(tnw[:], rs[:], rs[:])
                    nc.vector.tensor_mul(tnw[:], tnw[:], ss[:])
                    nc.vector.tensor_scalar(
                        tnw[:], tnw[:], -0.5, 1.5, ALU.mult, ALU.add
                    )
                    nc.vector.tensor_mul(rs[:], rs[:], tnw[:])

                for which, xs, off in (("q", xq, 0), ("k", xk, NTH)):
                    xn = qnp.tile(
                        [P, NTH * D], BF16, tag=f"xn{which}", name=f"xn{which}{h}_{hf}"
                    )
                    rs_b = rs[:, off : off + NTH].rearrange(
                        "p (t one) -> p t one", one=1
                    ).broadcast_to([P, NTH, D])
                    nc.vector.tensor_mul(
                        xn.rearrange("p (t d) -> p t d", d=D),
                        xs.rearrange("p (t d) -> p t d", d=D),
                        rs_b,
                    )
                    scratch = dramp.tile(
                        [HALF, D], BF16, tag=f"sc{which}", name=f"sc{which}{h}_{hf}"
                    )
                    dmae = nc.sync
                    dmae.dma_start(
                        out=scratch.rearrange("(t p) c -> p t c", p=P),
                        in_=xn.rearrange("p (t d) -> p t d", d=D),
                    )
                    xt = sd["qT"] if which == "q" else sd["kT"]
                    # read the scratch twice: duplicated partition halves
                    # feed the K=128 doubled-contraction matmuls
                    for dup in range(2):
                        nc.sync.dma_start_transpose(
                            out=xt[dup * D : (dup + 1) * D, c0 : c0 + HALF],
                            in_=scratch[:],
                        )
                    if which == "q":
                        nc.vector.tensor_scalar(
                            xt[:, c0 : c0 + HALF],
                            xt[:, c0 : c0 + HALF],
                            csc[:, 0:1],
                            None,
                            ALU.mult,
                        )

                vs = stagep.tile([P, NTH * D], FP32, tag="vs", name=f"vs{h}_{hf}")
                nc.sync.dma_start(
                    out=vs.rearrange("p (t d) -> p t d", d=D),
                    in_=v_d[h].rearrange("(t p) d -> p t d", p=P)[
                        :, tb : tb + NTH, :
                    ],
                )
                vb = sd["vb"]
                nc.vector.tensor_copy(
                    vb.rearrange("p (t c) -> p t c", c=D + 1)[
                        :, tb : tb + NTH, 0:D
                    ],
                    vs.rearrange("p (t d) -> p t d", d=D),
                )
                nc.gpsimd.memset(
                    vb.rearrange("p (t c) -> p t c", c=D + 1)[
                        :, tb : tb + NTH, D : D + 1
                    ],
                    1.0,
                )

            # ============ attention ============
            pending = []  # deferred epilogue emissions

            def run_pending():
                for f in pending:
                    f()
                pending.clear()

            def attention_half(h, ih):
                sd = st8[h]
                qT, kT, vb, mbias = sd["qT"], sd["kT"], sd["vb"], sd["mb"]
                ilo = ih * HALF
                ce = ilo + HALF
                njb = (ilo + HALF) // P  # 8 or 16
                oTh = otp.tile([D + 1, HALF], FP32, tag="oT", name=f"oT{h}_{ih}")
                pv_pending = None
                for jb in range(njb):
                    if jb == 2:
                        run_pending()
                    if ih == 0 and jb == 3:
                        pre_half(h, 1)
                    if ih == 1 and jb == 8 and h + 1 < HPC:
                        pre_half(h + 1, 0)
                    cs = max(jb * P, ilo)
                    W = ce - cs
                    has_diag = cs == jb * P
                    st = stp.tile([P, W], FP32, tag="st", name=f"st{h}_{ih}_{jb}")
                    if has_diag:
                        # bank-clearing first write: -1e30 strict-upper mask
                        # into the diagonal 128 cols; QK chunk 0 accumulates
                        nc.tensor.matmul(
                            st[:, 0:P],
                            trineg[:],
                            identb[:],
                            start=True,
                            stop=False,
                        )
                    n0 = cs
                    while n0 < ce:
                        w = min(512, ce - n0)
                        first = n0 == cs
                        nc.tensor.matmul(
                            st[:, n0 - cs : n0 - cs + w],
                            kT[:, jb * P : (jb + 1) * P],
                            qT[:, n0 : n0 + w],
                            start=not (has_diag and first),
                            stop=True,
                            skip_group_check=True,
                        )
                        n0 += w
                    pT = ptp.tile([P, W], BF16, tag="pT", name=f"pT{h}_{ih}_{jb}")
                    nc.scalar.activation(
                        pT[:],
                        st[:],
                        AF.Exp,
                        scale=COSINE_SIM_SCALE / 2.0,
                        bias=mbias[:, jb : jb + 1],
                    )
                    # PE software pipeline: PV for jb-1 lands AFTER QK of jb
                    if pv_pending is not None:
                        pv_pending()

                    def mk_pv(jb=jb, pT=pT, cs=cs):
                        def pv():
                            vslice = vb[:, jb * (D + 1) : (jb + 1) * (D + 1)]
                            n0 = cs
                            while n0 < ce:
                                rel = n0 - ilo
                                w = min(ilo + (rel // 512 + 1) * 512, ce) - n0
                                bank = rel // 512
                                last_jb = (ilo + 512 * bank + 511) // P
                                nc.tensor.matmul(
                                    oTh[:, rel : rel + w],
                                    vslice,
                                    pT[:, n0 - cs : n0 - cs + w],
                                    start=(jb == 0),
                                    stop=(jb == last_jb),
                                    skip_group_check=True,
                                )
                                n0 += w

                        return pv

                    pv_pending = mk_pv()
                pv_pending()

                # evacuate O^T now (frees the single otp slot for the next
                # half); the rest of the epilogue is deferred
                oT_sb = otsbp.tile([D + 1, HALF], FP32, tag="otsb", name=f"otsb{h}_{ih}")
                nc.vector.tensor_copy(oT_sb[:], oTh[:])

                def epi(h=h, ih=ih, oT_sb=oT_sb):
                    osb = osbp.tile([P, HALF // 2], FP32, tag="osb", name=f"osb{h}_{ih}")
                    for ib in range(NTH):
                        tp = stp.tile(
                            [P, D + 1], FP32, tag="st", name=f"tp{h}_{ih}_{ib}"
                        )
                        nc.tensor.transpose(
                            tp[:],
                            oT_sb[:, ib * P : (ib + 1) * P],
                            ident[0 : D + 1, 0 : D + 1],
                        )
                        rec = recp.tile([P, 1], FP32, tag="rec", name=f"rec{h}_{ih}_{ib}")
                        nc.vector.reciprocal(rec[:], tp[:, D : D + 1])
                        nc.vector.tensor_scalar(
                            osb[:, ib * D : (ib + 1) * D],
                            tp[:, 0:D],
                            rec[:, 0:1],
                            None,
                            ALU.mult,
                        )
                    nc.sync.dma_start(
                        out=out_d[h].rearrange("(t p) d -> p t d", p=P)[
                            :, ih * NTH : (ih + 1) * NTH, :
                        ],
                        in_=osb.rearrange("p (t d) -> p t d", d=D),
                    )

                pending.append(epi)

            pre_half(0, 0)
            for h in range(HPC):
                attention_half(h, 0)
                attention_half(h, 1)
            run_pending()

    nc.compile()
    return nc


_NC_CACHE = None


def kernel(q, k, v, q_scale, k_scale, mask):
    global _NC_CACHE
    q = np.asarray(q, dtype=np.float32)
    k = np.asarray(k, dtype=np.float32)
    v = np.asarray(v, dtype=np.float32)
    q_scale = np.asarray(q_scale, dtype=np.float32)
    k_scale = np.asarray(k_scale, dtype=np.float32)
    mask = np.asarray(mask)

    qf = q.reshape(B * H, S, D)
    kf = k.reshape(B * H, S, D)
    vf = v.reshape(B * H, S, D)
    # additive key-padding bias per (b,h) row, matching reference's where()
    mbias_bh = np.where(mask, 0.0, MASK_NEG).astype(np.float32)  # [B, S]

    if _NC_CACHE is None:
        _NC_CACHE = build_nc()
    nc = _NC_CACHE

    in_maps = []
    for c in range(N_CORES):
        heads = list(range(c * HPC, (c + 1) * HPC))
        in_maps.append(
            {
                "q": np.ascontiguousarray(qf[heads]),
                "k": np.ascontiguousarray(kf[heads]),
                "v": np.ascontiguousarray(vf[heads]),
                "q_scale": q_scale,
                "k_scale": k_scale,
                "mbias": np.ascontiguousarray(
                    np.stack([mbias_bh[bh // H] for bh in heads])
                ),
            }
        )

    res = run_bass_kernel_spmd(nc, in_maps, core_ids=list(range(N_CORES)))
    out = np.stack([r["out"] for r in res.results])  # [8, 4, S, D]
    return out.reshape(B, H, S, D).astype(np.float32)


# revision 4
# speedup vs baseline: 12.9004x; 12.9004x over previous
"""Causal cosine-sim attention (qk rmsnorm, scale=8) on 8 trn2 NeuronCores.

Shapes: q,k,v [2,16,2048,64] fp32; out [2,16,2048,64] fp32.
Sharding: 32 (batch, head) pairs -> 4 per core (head-parallel); each core
runs an identical SPMD program on its own 4 heads.

Per-core flash-attention-style algorithm (per head):
  preprocess (per 1024-token half, so the first matmul starts early and
    the next half overlaps attention): load Q (sync DMA) / K (gpsimd
    SWDGE); l2-normalize rows (squares on DVE for q / GpSimd for k,
    reduce + Quake-rsqrt + 2 Newton iters on DVE); cast bf16; round-trip
    through a DRAM scratch [1024,64] and DMA-xbar-transpose back TWICE
    into both 64-partition halves of qT/kT [128, s] (duplicated
    partition halves make the S^T matmuls contract over K=128: K=64
    matmuls don't count as PE-busy for the HAM activity monitor and pin
    the PE clock at 1.2 GHz).  q_scale*k_scale is folded into qT only.
    V loads [s,d] (gpsimd queue), cast bf16 with a ones-column appended
    (rowsum rides along in the PV matmul).
  attention (j-major over key blocks, i-halves of 1024): per key block
    jb, one S^T tile = kT_jb.T @ qT over the causal i-tail (PSUM,
    512-col matmuls); on diagonal blocks a constant strict-upper
    -1e30 matrix is matmul-accumulated into the first 128 cols BEFORE
    the exp (tri add via PE, so no GpSimd mask multiply serializes the
    PV chain); one ACT exp(4*x + mask_bias) per jb PSUM->SBUF bf16;
    O^T[65, i-half] += V_jb.T @ P^T accumulates in PSUM (col 64 =
    softmax denominator).  PV_{jb-1} is emitted AFTER QK_jb so the PE
    FIFO never head-of-line blocks on ScalarE's exp.
  epilogue per half: O^T PSUM->SBUF copy immediately (frees the PSUM
    bank for the next half); transposes (PE) + rowsum divide (DVE) +
    out-DMA (gpsimd queue) are deferred until the next half's pipeline
    is running so the PE never idles at half boundaries.
"""

import sys

import numpy as np

try:
    import concourse.bass as bass
except ImportError:
    sys.path.insert(0, "/opt/trn_rl_repo")
    import concourse.bass as bass

import concourse.mybir as mybir
import concourse.tile as tile
from concourse import bacc
from concourse.bass_utils import run_bass_kernel_spmd
from concourse.masks import make_identity

FP32 = mybir.dt.float32
BF16 = mybir.dt.bfloat16

N_CORES = 8
B, H, S, D = 2, 16, 2048, 64
HPC = (B * H) // N_CORES  # heads per core = 4
P = 128
NT = S // P  # 16 key/query blocks
HALF = S // 2
NTH = HALF // P  # 8 blocks per half
COSINE_SIM_SCALE = 8.0
MASK_NEG = -1e30


def build_nc():
    nc = bacc.Bacc("TRN2", target_bir_lowering=False, debug=False)

    q_d = nc.dram_tensor("q", [HPC, S, D], FP32, kind="ExternalInput")
    k_d = nc.dram_tensor("k", [HPC, S, D], FP32, kind="ExternalInput")
    v_d = nc.dram_tensor("v", [HPC, S, D], FP32, kind="ExternalInput")
    qs_d = nc.dram_tensor("q_scale", [D], FP32, kind="ExternalInput")
    ks_d = nc.dram_tensor("k_scale", [D], FP32, kind="ExternalInput")
    mb_d = nc.dram_tensor("mbias", [HPC, S], FP32, kind="ExternalInput")
    out_d = nc.dram_tensor("out", [HPC, S, D], FP32, kind="ExternalOutput")

    AF = mybir.ActivationFunctionType
    ALU = mybir.AluOpType

    with tile.TileContext(nc) as tc:
        with (
            tc.tile_pool(name="constp", bufs=1) as constp,
            tc.tile_pool(name="dramp", bufs=2, space="DRAM") as dramp,
            tc.tile_pool(name="stagep", bufs=3) as stagep,
            tc.tile_pool(name="sqp", bufs=2) as sqp,
            tc.tile_pool(name="ssp", bufs=6) as ssp,
            tc.tile_pool(name="qnp", bufs=3) as qnp,
            tc.tile_pool(name="qtp", bufs=2) as qtp,
            tc.tile_pool(name="ktp", bufs=2) as ktp,
            tc.tile_pool(name="vbp", bufs=2) as vbp,
            tc.tile_pool(name="mbp", bufs=2) as mbp,
            tc.tile_pool(name="ptp", bufs=8) as ptp,
            tc.tile_pool(name="otsbp", bufs=2) as otsbp,
            tc.tile_pool(name="osbp", bufs=2) as osbp,
            tc.tile_pool(name="recp", bufs=8) as recp,
            tc.tile_pool(name="stp", bufs=3, space="PSUM") as stp,
            tc.tile_pool(name="otp", bufs=1, space="PSUM") as otp,
        ):
            # ---- constants ----
            ident = constp.tile([P, P], FP32, name="ident")
            make_identity(nc, ident[:])
            identb = constp.tile([P, P], BF16, name="identb")
            nc.vector.tensor_copy(identb[:], ident[:])
            # trineg[row=i, col=j] = -1e30 where j > i (strict upper).
            # Used as lhsT in a matmul-accumulate against identb:
            # st[j, i] += trineg[i, j] masks j > i before the exp.
            trineg = constp.tile([P, P], BF16, name="trineg")
            nc.gpsimd.memset(trineg[:], MASK_NEG)
            nc.gpsimd.affine_select(
                out=trineg[:],
                in_=trineg[:],
                pattern=[[1, P]],
                channel_multiplier=-1,
                base=-1,
                compare_op=ALU.is_ge,
                fill=0.0,
            )
            # combined q_scale*k_scale (applied to qT only), duplicated
            # over both partition halves to match the row-packed qT
            csc = constp.tile([P, 1], FP32, name="csc")
            kss = constp.tile([P, 1], FP32, name="kss")
            for half in range(2):
                nc.sync.dma_start(
                    out=csc[half * D : (half + 1) * D, 0:1],
                    in_=qs_d[:].rearrange("(d one) -> d one", one=1),
                )
                nc.sync.dma_start(
                    out=kss[half * D : (half + 1) * D, 0:1],
                    in_=ks_d[:].rearrange("(d one) -> d one", one=1),
                )
            nc.vector.tensor_mul(csc[:], csc[:], kss[:])

            # ============ per-(head, half) preprocess ============
            st8 = {}  # h -> dict(qT, kT, vb, mb)

            def pre_half(h, hf):
                c0 = hf * HALF
                tb = hf * NTH  # first 128-token block of this half
                if hf == 0:
                    st8[h] = {
                        "qT": qtp.tile([P, S], BF16, tag="qT", name=f"qT{h}"),
                        "kT": ktp.tile([P, S], BF16, tag="kT", name=f"kT{h}"),
                        "vb": vbp.tile(
                            [P, NT * (D + 1)], BF16, tag="vb", name=f"vb{h}"
                        ),
                        "mb": mbp.tile([P, NT], FP32, tag="mb", name=f"mb{h}"),
                    }
                    nc.sync.dma_start(
                        out=st8[h]["mb"][:],
                        in_=mb_d[h].rearrange("(t p) -> p t", p=P),
                    )
                sd = st8[h]

                xq = stagep.tile([P, NTH * D], FP32, tag="xq", name=f"xq{h}_{hf}")
                nc.sync.dma_start(
                    out=xq.rearrange("p (t d) -> p t d", d=D),
                    in_=q_d[h].rearrange("(t p) d -> p t d", p=P)[
                        :, tb : tb + NTH, :
                    ],
                )
                xk = stagep.tile([P, NTH * D], FP32, tag="xk", name=f"xk{h}_{hf}")
                nc.sync.dma_start(
                    out=xk.rearrange("p (t d) -> p t d", d=D),
                    in_=k_d[h].rearrange("(t p) d -> p t d", p=P)[
                        :, tb : tb + NTH, :
                    ],
                )
                # sum-of-squares: q on DVE, k squares on GpSimd (reduce
                # must be DVE: GpSimd cannot reduce along the free axis)
                sqq = sqp.tile([P, NTH * D], FP32, tag="sqq", name=f"sqq{h}_{hf}")
                nc.vector.tensor_mul(sqq[:], xq[:], xq[:])
                sqk = sqp.tile([P, NTH * D], FP32, tag="sqk", name=f"sqk{h}_{hf}")
                nc.gpsimd.tensor_mul(sqk[:], xk[:], xk[:])
                ss = ssp.tile([P, 2 * NTH], FP32, tag="ss", name=f"ss{h}_{hf}")
                nc.vector.tensor_reduce(
                    out=ss[:, 0:NTH],
                    in_=sqq.rearrange("p (t d) -> p t d", d=D),
                    axis=mybir.AxisListType.X,
                    op=ALU.add,
                )
                nc.vector.tensor_reduce(
                    out=ss[:, NTH : 2 * NTH],
                    in_=sqk.rearrange("p (t d) -> p t d", d=D),
                    axis=mybir.AxisListType.X,
                    op=ALU.add,
                )
                # rsqrt(ss) on DVE: Quake magic init + 2 Newton iterations
                rs = ssp.tile([P, 2 * NTH], FP32, tag="ss", name=f"rs{h}_{hf}")
                rsi = rs.bitcast(mybir.dt.int32)
                nc.vector.tensor_scalar(
                    rsi, ss.bitcast(mybir.dt.int32), 1, None, ALU.arith_shift_right
                )
                nc.vector.tensor_scalar(
                    rsi, rsi, -1.0, float(0x5F3759DF), ALU.mult, ALU.add
                )
                tnw = ssp.tile([P, 2 * NTH], FP32, tag="ss", name=f"tnw{h}_{hf}")
                for _ in range(2):
                    nc.vector.tensor_mul(tnw[:], rs[:], rs[:])
                    nc.vector.tensor_mul(tnw[:], tnw[:], ss[:])
                    nc.vector.tensor_scalar(
                        tnw[:], tnw[:], -0.5, 1.5, ALU.mult, ALU.add
                    )
                    nc.vector.tensor_mul(rs[:], rs[:], tnw[:])

                for which, xs, off in (("q", xq, 0), ("k", xk, NTH)):
                    xn = qnp.tile(
                        [P, NTH * D], BF16, tag=f"xn{which}", name=f"xn{which}{h}_{hf}"
                    )
                    rs_b = rs[:, off : off + NTH].rearrange(
                        "p (t one) -> p t one", one=1
                    ).broadcast_to([P, NTH, D])
                    nc.vector.tensor_mul(
                        xn.rearrange("p (t d) -> p t d", d=D),
                        xs.rearrange("p (t d) -> p t d", d=D),
                        rs_b,
                    )
                    # scratch is [HALF, 128] (xbar needs free dim >= 128);
                    # both 64-col halves carry the same data: the duplicate
                    # feeds the K=128 doubled-contraction matmuls
                    scratch = dramp.tile(
                        [HALF, P], BF16, tag=f"sc{which}", name=f"sc{which}{h}_{hf}"
                    )
                    for dup in range(2):
                        nc.sync.dma_start(
                            out=scratch.rearrange("(t p) c -> p t c", p=P)[
                                :, :, dup * D : (dup + 1) * D
                            ],
                            in_=xn.rearrange("p (t d) -> p t d", d=D),
                        )
                    xt = sd["qT"] if which == "q" else sd["kT"]
                    nc.sync.dma_start_transpose(
                        out=xt[:, c0 : c0 + HALF], in_=scratch[:]
                    )
                    if which == "q":
                        nc.vector.tensor_scalar(
                            xt[:, c0 : c0 + HALF],
                            xt[:, c0 : c0 + HALF],
                            csc[:, 0:1],
                            None,
                            ALU.mult,
                        )

                vs = stagep.tile([P, NTH * D], FP32, tag="vs", name=f"vs{h}_{hf}")
                nc.sync.dma_start(
                    out=vs.rearrange("p (t d) -> p t d", d=D),
                    in_=v_d[h].rearrange("(t p) d -> p t d", p=P)[
                        :, tb : tb + NTH, :
                    ],
                )
                vb = sd["vb"]
                nc.vector.tensor_copy(
                    vb.rearrange("p (t c) -> p t c", c=D + 1)[
                        :, tb : tb + NTH, 0:D
                    ],
                    vs.rearrange("p (t d) -> p t d", d=D),
                )
                nc.gpsimd.memset(
                    vb.rearrange("p (t c) -> p t c", c=D + 1)[
                        :, tb : tb + NTH, D : D + 1
                    ],
                    1.0,
                )

            # ============ attention ============
            pending = []  # deferred epilogue emissions

            def run_pending():
                for f in pending:
                    f()
                pending.clear()

            def attention_half(h, ih):
                sd = st8[h]
                qT, kT, vb, mbias = sd["qT"], sd["kT"], sd["vb"], sd["mb"]
                ilo = ih * HALF
                ce = ilo + HALF
                njb = (ilo + HALF) // P  # 8 or 16
                oTh = otp.tile([D + 1, HALF], FP32, tag="oT", name=f"oT{h}_{ih}")
                pv_pending = None
                for jb in range(njb):
                    if jb == 2:
                        run_pending()
                    if ih == 0 and jb == 3:
                        pre_half(h, 1)
                    if ih == 1 and jb == 8 and h + 1 < HPC:
                        pre_half(h + 1, 0)
                    cs = max(jb * P, ilo)
                    W = ce - cs
                    has_diag = cs == jb * P
                    st = stp.tile([P, W], FP32, tag="st", name=f"st{h}_{ih}_{jb}")
                    if has_diag:
                        # bank-clearing first write: -1e30 strict-upper mask
                        # into the diagonal 128 cols; QK chunk 0 accumulates
                        nc.tensor.matmul(
                            st[:, 0:P],
                            trineg[:],
                            identb[:],
                            start=True,
                            stop=False,
                        )
                    n0 = cs
                    while n0 < ce:
                        w = min(512, ce - n0)
                        first = n0 == cs
                        nc.tensor.matmul(
                            st[:, n0 - cs : n0 - cs + w],
                            kT[:, jb * P : (jb + 1) * P],
                            qT[:, n0 : n0 + w],
                            start=not (has_diag and first),
                            stop=True,
                            skip_group_check=True,
                        )
                        n0 += w
                    pT = ptp.tile([P, W], BF16, tag="pT", name=f"pT{h}_{ih}_{jb}")
                    nc.scalar.activation(
                        pT[:],
                        st[:],
                        AF.Exp,
                        scale=COSINE_SIM_SCALE / 2.0,
                        bias=mbias[:, jb : jb + 1],
                    )
                    # PE software pipeline: PV for jb-1 lands AFTER QK of jb
                    if pv_pending is not None:
                        pv_pending()

                    def mk_pv(jb=jb, pT=pT, cs=cs):
                        def pv():
                            vslice = vb[:, jb * (D + 1) : (jb + 1) * (D + 1)]
                            n0 = cs
                            while n0 < ce:
                                rel = n0 - ilo
                                w = min(ilo + (rel // 512 + 1) * 512, ce) - n0
                                bank = rel // 512
                                last_jb = (ilo + 512 * bank + 511) // P
                                nc.tensor.matmul(
                                    oTh[:, rel : rel + w],
                                    vslice,
                                    pT[:, n0 - cs : n0 - cs + w],
                                    start=(jb == 0),
                                    stop=(jb == last_jb),
                                    skip_group_check=True,
                                )
                                n0 += w

                        return pv

                    pv_pending = mk_pv()
                pv_pending()

                # evacuate O^T now (frees the single otp slot for the next
                # half); the rest of the epilogue is deferred
                oT_sb = otsbp.tile([D + 1, HALF], FP32, tag="otsb", name=f"otsb{h}_{ih}")
                nc.vector.tensor_copy(oT_sb[:], oTh[:])

                def epi(h=h, ih=ih, oT_sb=oT_sb):
                    osb = osbp.tile([P, HALF // 2], FP32, tag="osb", name=f"osb{h}_{ih}")
                    for ib in range(NTH):
                        tp = stp.tile(
                            [P, D + 1], FP32, tag="st", name=f"tp{h}_{ih}_{ib}"
                        )
                        nc.tensor.transpose(
                            tp[:],
                            oT_sb[:, ib * P : (ib + 1) * P],
                            ident[0 : D + 1, 0 : D + 1],
                        )
                        rec = recp.tile([P, 1], FP32, tag="rec", name=f"rec{h}_{ih}_{ib}")
                        nc.vector.reciprocal(rec[:], tp[:, D : D + 1])
                        nc.vector.tensor_scalar(
                            osb[:, ib * D : (ib + 1) * D],
                            tp[:, 0:D],
                            rec[:, 0:1],
                            None,
                            ALU.mult,
                        )
                    nc.sync.dma_start(
                        out=out_d[h].rearrange("(t p) d -> p t d", p=P)[
                            :, ih * NTH : (ih + 1) * NTH, :
                        ],
                        in_=osb.rearrange("p (t d) -> p t d", d=D),
                    )

                pending.append(epi)

            pre_half(0, 0)
            for h in range(HPC):
                attention_half(h, 0)
                attention_half(h, 1)
            run_pending()

    nc.compile()
    return nc


_NC_CACHE = None


def kernel(q, k, v, q_scale, k_scale, mask):
    global _NC_CACHE
    q = np.asarray(q, dtype=np.float32)
    k = np.asarray(k, dtype=np.float32)
    v = np.asarray(v, dtype=np.float32)
    q_scale = np.asarray(q_scale, dtype=np.float32)
    k_scale = np.asarray(k_scale, dtype=np.float32)
    mask = np.asarray(mask)

    qf = q.reshape(B * H, S, D)
    kf = k.reshape(B * H, S, D)
    vf = v.reshape(B * H, S, D)
    # additive key-padding bias per (b,h) row, matching reference's where()
    mbias_bh = np.where(mask, 0.0, MASK_NEG).astype(np.float32)  # [B, S]

    if _NC_CACHE is None:
        _NC_CACHE = build_nc()
    nc = _NC_CACHE

    in_maps = []
    for c in range(N_CORES):
        heads = list(range(c * HPC, (c + 1) * HPC))
        in_maps.append(
            {
                "q": np.ascontiguousarray(qf[heads]),
                "k": np.ascontiguousarray(kf[heads]),
                "v": np.ascontiguousarray(vf[heads]),
                "q_scale": q_scale,
                "k_scale": k_scale,
                "mbias": np.ascontiguousarray(
                    np.stack([mbias_bh[bh // H] for bh in heads])
                ),
            }
        )

    res = run_bass_kernel_spmd(nc, in_maps, core_ids=list(range(N_CORES)))
    out = np.stack([r["out"] for r in res.results])  # [8, 4, S, D]
    return out.reshape(B, H, S, D).astype(np.float32)
